# revision 26
# baseline (speedup 1.0000x reference)
"""Trainium2 Bass kernel v2 for gated multi-head attention with additive bias.

Reference (b=2, n=2048, dim=256, h=8, dh=32):
    q = x @ Wq;  k,v = split(x @ Wkv);  dots = q k^T / sqrt(dh) + attn_bias
    attn = softmax(dots);  out = attn @ v
    out = out * sigmoid(x @ Wg + bg);  return out @ Wout + bout

Sharding: 16 (batch, head) pairs -> 8 cores, 2 heads each (core c handles
batch c//4, heads 2*(c%4), 2*(c%4)+1).  Host ships exp(bias)^T in bf16 and
sums the per-core partial outputs.

v2 design (vs the v1 baseline):
  * S^T chunks land in a manual 3-slot PSUM ring (each slot [128,1024] = 2
    banks).  exp() reads PAIRS of adjacent slots as one wide [128,2048] ACT
    instruction when the ring doesn't wrap (2 of 3 pairs) -- ACT is the
    bottleneck engine and its (N+352)-cycle cost rewards wide reads.
  * attn@v accumulates into ONE bank per i-half with M=33 blocks at psum
    base partitions 0 and 64 (tile_position col packing) -- 2 banks total,
    leaving 6 banks for the S ring.  The 33rd v-column of ones produces
    softmax row sums in psum rows 32/96.
  * q/k for both heads are computed with a single fused M=128 weight
    (columns = [q_h0|q_h1|k_h0|k_h1]), gates in the attn@v banks, so the PE
    stream is dense from t=0 (HAM stays un-throttled at 2.4 GHz).
  * Normalization deferred past Wout: row sums bounce through DRAM to
    become per-partition scalars [128, nt]; reciprocal multiplies the
    projection output.  Head 0's projection is interleaved into head 1's
    main loop using just-freed ring slots.

Toolchain: walrus accepts at most ONE semaphore wait per compute-engine
instruction; _split_multi_waits moves extras onto same-engine NOPs.
"""

import os
import sys

import numpy as np

for _p in ("/opt/trn_rl_repo", "/root/.axon_site/_ro/trn_rl_repo"):
    if os.path.isdir(_p) and _p not in sys.path:
        sys.path.insert(0, _p)

B = 2
N = 2048
DIM = 256
HEADS = 8
DH = 32
HPC = 2
NCORES = 8
P = 128
NT = N // P          # 16 j-tiles
NQ = N // 512        # 4 query chunks of 512


def build_nc(split_waits=True):
    import concourse.bass as bass
    import concourse.mybir as mybir
    from concourse.bass import ts
    from concourse.tile import TileContext

    f32 = mybir.dt.float32
    bf16 = mybir.dt.bfloat16
    Act = mybir.ActivationFunctionType

    n, dim, nt = N, DIM, NT
    nck = dim // P               # 2 contraction chunks over model dim
    cw = const_width()

    from concourse import tile_sem_assignment as _tsa

    _swdge_prev = _tsa.NUM_SWDGE_GLOBAL_SEMS

    nc = bass.Bass()

    cb = nc.declare_dram_parameter("cb", [P, cw], bf16, isOutput=False)
    biasT = nc.declare_dram_parameter("biasT", [HPC, n, n], bf16, isOutput=False)
    out_ext = nc.declare_dram_parameter("out", [HPC, n, dim], bf16, isOutput=True)

    _tsa.NUM_SWDGE_GLOBAL_SEMS = 1
    with TileContext(nc) as tc:
        with (
            tc.tile_pool(name="consts", bufs=1) as consts,
            tc.tile_pool(name="dram", bufs=2, space="DRAM") as dpool,
            tc.tile_pool(name="s_ps", bufs=3, space="PSUM") as spool,
            tc.tile_pool(name="av_ps", bufs=1, space="PSUM") as avpool,
            tc.tile_pool(name="bias", bufs=4) as bpool,
            tc.tile_pool(name="attn", bufs=7) as apool,
            tc.tile_pool(name="et", bufs=3) as etpool,
        ):
            # ---- packed constant DMA, split so the small weights and the
            # x half-0 blocks land first (prologue critical path) ----
            cb_sb = consts.tile([P, cw], bf16, tag="cb", name="cb_sb")
            xw = nck * n                      # 4096 cols of xT, c-major
            nc.sync.dma_start(out=cb_sb[:, xw:cw], in_=cb[:, xw:cw])
            for rng in ((0, 1024), (2048, 3072), (1024, 2048), (3072, 4096)):
                nc.sync.dma_start(out=cb_sb[:, rng[0] : rng[1]],
                                  in_=cb[:, rng[0] : rng[1]])
            off = 0

            def take(cols):
                nonlocal off
                ap = cb_sb[:, off : off + cols]
                off += cols
                return ap

            xT_sb = take(nck * n).rearrange("p (c n) -> p c n", c=nck)
            # per chunk c, cols = [q_h0|q_h1|k_h0|k_h1] x 32
            wqk = take(nck * P).rearrange("p (c m) -> p c m", c=nck)
            # gate weights: per chunk c, cols = [g_h0|g_h1] x 32
            wgg = take(nck * 2 * DH).rearrange("p (c m) -> p c m", c=nck)
            # v weights: per chunk c, cols = [v_h0|v_h1] x 32
            wvv = take(nck * 2 * DH).rearrange("p (c m) -> p c m", c=nck)
            # wout replicated on partition rows 0-31 and 64-95
            wout_cols = take(HPC * dim)
            wout_h = [wout_cols[0:DH, h * dim : (h + 1) * dim] for h in range(HPC)]
            wout_h_odd = [wout_cols[64 : 64 + DH, h * dim : (h + 1) * dim]
                          for h in range(HPC)]
            bg_cols = take(HPC)
            bg_h = [bg_cols[0:DH, h : h + 1] for h in range(HPC)]
            eye_cols = take(16)
            eye16 = eye_cols[0:16, :]
            assert off == cw

            # ---- persistent SBUF ----
            qT_h = [consts.tile([P, n], bf16, tag=f"qT{h}", name=f"qT{h}")
                    for h in range(HPC)]
            kT_h = [consts.tile([P, n], bf16, tag=f"kT{h}", name=f"kT{h}")
                    for h in range(HPC)]
            gT_h = [consts.tile([DH + 1, n], bf16, tag=f"gT{h}", name=f"gT{h}")
                    for h in range(HPC)]
            gT_odd = [consts.tile([97, n], bf16, tag=f"gTo{h}", name=f"gTo{h}")
                      for h in range(HPC)]
            vtmp = consts.tile([P, nt, 2 * DH], bf16, tag="vtmp", name="vtmp")
            # per head: [v_h | ones] (33 cols) per j-tile
            vaug = consts.tile([P, nt, HPC, DH + 1], bf16, tag="vaug", name="vaug")
            gatedT_h = [consts.tile([DH + 1, n], bf16, tag=f"gatedT{h}",
                                    name=f"gatedT{h}") for h in range(HPC)]
            godT_h = [consts.tile([97, n], bf16, tag=f"godT{h}", name=f"godT{h}")
                      for h in range(HPC)]
            recip_h = [consts.tile([P, nt], f32, tag=f"recip{h}",
                                   name=f"recip{h}") for h in range(HPC)]
            o_all = [consts.tile([P, nt, dim], bf16, tag=f"o{h}", name=f"o{h}")
                     for h in range(HPC)]

            zrow = consts.tile([1, 512], bf16, tag="zrow", name="zrow")
            nc.vector.memset(zrow, 0.0)

            # ---- PSUM: 3-slot S pool (6 banks) + attn@v banks (2) ----
            SPAD = [P, 1024]
            av = avpool.tile([P, 1024], f32, tag="av", name="av")



            # =========== prologue ===========
            # q/k: col-tiled pair per strip (q_h0 -> rows 0-31, q_h1 ->
            # rows 32-63 concurrently), evacuated as one aligned [64, .] copy
            def emit_qk(i, half):
                # i: 0 = q (heads 0/1), 1 = k (heads 0/1).  The two heads go
                # to different pool slots (distinct banks) at col groups 0/1,
                # so the matmul pairs run concurrently on the PE.
                qkA = spool.tile([DH, 1024], f32, tag="s", name="qkA",
                                 padded_shape=SPAD)
                qkB = spool.tile([2 * DH, 1024], f32, tag="s", name="qkB",
                                 padded_shape=SPAD)
                dsts = (qT_h, kT_h)[i]
                for c in range(nck):
                    for s in range(2):
                        for e in range(2):
                            out = (qkA[0:DH, ts(s, 512)] if e == 0
                                   else qkB[DH : 2 * DH, ts(s, 512)])
                            nc.tensor.matmul(
                                out,
                                wqk[:, c, (2 * i + e) * DH :
                                    (2 * i + e + 1) * DH],
                                xT_sb[:, c, half * 1024 + s * 512 :
                                      half * 1024 + (s + 1) * 512],
                                start=(c == 0),
                                stop=(c == nck - 1),
                            )
                cols = slice(half * 1024, (half + 1) * 1024)
                nc.vector.tensor_copy(dsts[0][0:DH, cols], qkA[0:DH, :])
                nc.vector.tensor_copy(dsts[1][DH : 2 * DH, cols],
                                      qkB[DH : 2 * DH, :])

            def replicate_qk(h):
                # fill the remaining 3 row groups of each [128, n] tile
                nc.sync.dma_start(out=qT_h[h][DH : 2 * DH, :] if h == 0
                                  else qT_h[h][0:DH, :],
                                  in_=qT_h[h][0:DH, :] if h == 0
                                  else qT_h[h][DH : 2 * DH, :])
                nc.sync.dma_start(out=kT_h[h][DH : 2 * DH, :] if h == 0
                                  else kT_h[h][0:DH, :],
                                  in_=kT_h[h][0:DH, :] if h == 0
                                  else kT_h[h][DH : 2 * DH, :])
                for tile in (qT_h[h], kT_h[h]):
                    nc.sync.dma_start(out=tile[2 * DH : 4 * DH, :],
                                      in_=tile[0 : 2 * DH, :])

            def emit_v(t):
                vps = spool.tile([P, 64], f32, tag="s", name="vps",
                                 padded_shape=SPAD)
                for c in range(nck):
                    nc.tensor.matmul(
                        vps,
                        xT_sb[:, c, ts(t, P)],
                        wvv[:, c, :],
                        start=(c == 0),
                        stop=(c == nck - 1),
                    )
                nc.vector.tensor_copy(vtmp[:, t, :], vps)

            # q/k first: the main loop's S matmuls gate on these
            for i in (0, 1):
                for half in range(2):
                    emit_qk(i, half)
            replicate_qk(0)
            replicate_qk(1)
            # gates: [32, 512] psums; sigmoids run on ACT before the first exp
            for h in range(HPC):
                for s in range(4):
                    gps = spool.tile([DH, 512], f32, tag="s", name="gps",
                                     padded_shape=SPAD)
                    for c in range(nck):
                        nc.tensor.matmul(
                            gps,
                            wgg[:, c, h * DH : (h + 1) * DH],
                            xT_sb[:, c, ts(s, 512)],
                            start=(c == 0),
                            stop=(c == nck - 1),
                        )
                    nc.scalar.activation(
                        out=gT_h[h][0:DH, ts(s, 512)],
                        in_=gps,
                        func=Act.Sigmoid,
                        scale=1.0,
                        bias=bg_h[h],
                    )
            for h in range(HPC):
                nc.vector.memset(gT_h[h][DH : DH + 1, :], 1.0)
                nc.sync.dma_start(out=gT_odd[h][64:97, :], in_=gT_h[h][0:33, :])

            # =========== main loops ===========
            def emit_proj(h, t):
                """Projection for (head h, i-tile t) through a transient psum
                strip; normalized output lands in o_all[h]."""
                pp = spool.tile([P, dim], f32, tag="s", name="pp",
                                padded_shape=SPAD)
                if (t // 4) % 2 == 0:
                    lhsT = gatedT_h[h][0:DH, ts(t, P)]
                    rhs = wout_h[h]
                else:
                    lhsT = godT_h[h][64 : 64 + DH, ts(t, P)]
                    rhs = wout_h_odd[h]
                nc.tensor.matmul(pp, lhsT, rhs, start=True, stop=True)
                nc.vector.tensor_scalar_mul(
                    o_all[h][:, t, :], pp, recip_h[h][:, t : t + 1]
                )

            PROJ_ORDER = [0, 4, 1, 5, 2, 6, 3, 7, 8, 12, 9, 13, 10, 14,
                          11, 15]
            cc = 0
            for h in range(HPC):
                # open the two attn@v banks lazily at the first AV flush:
                # K=1 zero outer-products write all 128 partitions with
                # start=True.  Deferred evacuation reads of the PREVIOUS
                # head's results must precede these writes in emission order.
                opened = [False]

                def open_banks():
                    for bank in range(2):
                        nc.tensor.matmul(
                            av[:, 512 * bank : 512 * bank + 512],
                            zrow[0:1, 0:P],
                            zrow[0:1, 0:512],
                            start=True,
                            stop=False,
                            skip_group_check=True,
                        )
                    opened[0] = True
                def emit_av(jc_, m_, attn_):
                    # attn@v: M=33 blocks at psum partitions 0/64 of bank m_
                    for e in range(2):
                        nc.tensor.matmul(
                            av[64 * e : 64 * e + DH + 1,
                               512 * m_ : 512 * m_ + 512],
                            vaug[:, jc_, h, :],
                            attn_[:, ts(e, 512)],
                            start=False,
                            stop=(jc_ == nt - 1 and e == 1),
                            skip_group_check=True,
                        )

                pend = []
                for jc in range(nt):
                    bt = bpool.tile([P, n], bf16, tag="bias", name="bt")
                    nc.sync.dma_start(out=bt, in_=biasT[h, ts(jc, P), :])
                    # two 1024-wide chunks per j-tile; AV emission lags two
                    # chunks so the in-order PE queue never waits on exp/mul
                    for m in range(2):
                        sps = spool.tile([P, 1024], f32, tag="s", name="sps")
                        for s in range(2):
                            g = (2 * cc + s) % 4
                            nc.tensor.matmul(
                                sps[:, ts(s, 512)],
                                kT_h[h][32 * g : 32 * (g + 1), ts(jc, P)],
                                qT_h[h][32 * g : 32 * (g + 1),
                                        m * 1024 + s * 512 :
                                        m * 1024 + (s + 1) * 512],
                                start=True,
                                stop=True,
                                tile_position=(32 * g, 0),
                            )
                        cc += 1
                        et = etpool.tile([P, 1024], bf16, tag="et", name="et")
                        nc.scalar.activation(out=et, in_=sps, func=Act.Exp)
                        attn = apool.tile([P, 1024], bf16, tag="attn",
                                          name="attn")
                        nc.vector.tensor_mul(
                            attn, et, bt[:, m * 1024 : (m + 1) * 1024]
                        )
                        if h == 0 and jc == 0 and m == 0:
                            # v + vaug assembly rides the pipe-fill bubble:
                            # emitted before AV(0) enters the in-order PE queue
                            nc.vector.memset(vaug[:, :, :, DH : DH + 1], 1.0)
                            for t in range(nt):
                                emit_v(t)
                            nc.vector.tensor_copy(
                                vaug[:, :, :, 0:DH],
                                vtmp.rearrange("p t (h d) -> p t h d", h=HPC),
                            )
                        pend.append((jc, m, attn))
                    # flush attn@v two chunks at a time: consecutive same-type
                    # matmul runs keep LDWEIGHTS hidden under the streams
                    while len(pend) > 4:
                        if not opened[0]:
                            open_banks()
                        emit_av(*pend.pop(0))
                        emit_av(*pend.pop(0))
                    if h == 1 and jc <= 1:
                        deferred_gated(2 * jc)
                        deferred_gated(2 * jc + 1)
                    if h == 1 and jc in (2, 3):
                        deferred_strip(2 * (jc - 2))
                        deferred_strip(2 * (jc - 2) + 1)
                    if h == 1 and jc == 4:
                        deferred_recip()
                    # interleave head-0 projection into head-1's loop
                    # (order alternates row groups 0/2 so proj pairs overlap)
                    if h == 1 and 5 <= jc:
                        t2 = PROJ_ORDER[jc - 5]
                        emit_proj(0, t2)
                        nc.sync.dma_start(
                            out=out_ext[0, t2 * P : (t2 + 1) * P, :],
                            in_=o_all[0][:, t2, :],
                        )
                for a in pend:
                    emit_av(*a)

                # ---- evacuate attn@v psum for this head ----
                def emit_gated(q, h_=h):
                    qcol = 512 * (q // 2)
                    if q % 2 == 0:
                        nc.vector.tensor_mul(
                            gatedT_h[h_][:, ts(q, 512)],
                            av[0 : DH + 1, qcol : qcol + 512],
                            gT_h[h_][:, ts(q, 512)],
                        )
                    else:
                        nc.vector.tensor_mul(
                            godT_h[h_][64:97, ts(q, 512)],
                            av[64 : 64 + DH + 1, qcol : qcol + 512],
                            gT_odd[h_][64:97, ts(q, 512)],
                        )

                if h == 0:
                    deferred_gated = emit_gated
                else:
                    for q in range(NQ):
                        emit_gated(q)
                # ones-row of gates turned the psum sums rows into row DH of
                # the gated tiles; bounce through DRAM to [128, nt]
                scr = dpool.tile([n], bf16, tag="scr", name="scr")

                def emit_strip(q, h_=h, scr_=scr):
                    if q % 2 == 0:
                        strip = gatedT_h[h_][DH : DH + 1, ts(q, 512)]
                    else:
                        strip = godT_h[h_][96:97, ts(q, 512)]
                    nc.sync.dma_start(out=scr_[ts(q, 512)], in_=strip)

                if h == 0:
                    deferred_strip = emit_strip
                else:
                    for q in range(NQ):
                        emit_strip(q)
                def emit_recip(h_=h, scr_=scr):
                    sums16 = consts.tile([nt, P], bf16, tag=f"sums16{h_}",
                                         name=f"sums16{h_}")
                    nc.sync.dma_start(
                        out=sums16, in_=scr_[:].rearrange("(t p) -> t p", p=P)
                    )
                    spt = spool.tile([P, nt], f32, tag="s", name="spt",
                                     padded_shape=SPAD)
                    nc.tensor.matmul(spt, sums16, eye16, start=True, stop=True)
                    nc.vector.reciprocal(recip_h[h_], spt)

                if h == 0:
                    # defer: keeps the transpose matmul out of the in-order PE
                    # queue ahead of head 1's S chunks
                    deferred_recip = emit_recip
                else:
                    emit_recip()

            # remaining projections + head-1 epilogue
            for k in range(nt - 5, nt):
                t2 = PROJ_ORDER[k]
                emit_proj(0, t2)
                nc.sync.dma_start(
                    out=out_ext[0, t2 * P : (t2 + 1) * P, :],
                    in_=o_all[0][:, t2, :],
                )
            for t in range(nt):
                t2 = PROJ_ORDER[t]
                emit_proj(1, t2)
                nc.sync.dma_start(
                    out=out_ext[1, t2 * P : (t2 + 1) * P, :],
                    in_=o_all[1][:, t2, :],
                )

    _tsa.NUM_SWDGE_GLOBAL_SEMS = _swdge_prev
    if split_waits:
        _split_multi_waits(nc)
    return nc


def _split_multi_waits(nc):
    """walrus accepts at most ONE semaphore wait per engine instruction;
    extra waits ride same-engine NOPs inserted just before (queues execute
    in order)."""
    import concourse.mybir as mybir

    n = 0
    for f in nc.m.functions:
        for blk in f.blocks:
            out = []
            changed = False
            for inst in blk.instructions:
                si = getattr(inst, "sync_info", None)
                ws = list(si.on_wait) if si and si.on_wait else []
                if len(ws) > 1:
                    for w in ws[:-1]:
                        nop = mybir.InstNoOp(
                            name=f"I-waitsplit-{n}",
                            engine=inst.engine,
                            sync_info=mybir.SyncInfo(on_wait=[w], on_update=[]),
                        )
                        out.append(nop)
                        n += 1
                    si.on_wait = [ws[-1]]
                    inst.sync_info = si
                    changed = True
                out.append(inst)
            if changed:
                blk.instructions = out


def check_mm_waits(nc):
    bad = []
    for f in nc.m.functions:
        for blk in f.blocks:
            for inst in blk.instructions:
                if type(inst).__name__ in ("InstDMACopy", "InstDrain"):
                    continue
                si = getattr(inst, "sync_info", None)
                ws = list(si.on_wait) if si and si.on_wait else []
                if len(ws) > 1:
                    bad.append(
                        (inst.name, type(inst).__name__,
                         [(w.ant_name, w.wait_value) for w in ws])
                    )
    return bad


def const_width():
    nck = DIM // P
    return (nck * N + nck * P + nck * 2 * DH + nck * 2 * DH + HPC * DIM
            + HPC + 16)


def pack_consts(xT, wq2, wk2, wv2, wg2, bg2, wout2):
    """xT [dim, n]; wq2/wk2/wv2/wg2 [dim, 2*DH] (head-major cols);
    bg2 [2*DH]; wout2 [2*DH, dim]."""
    nck = DIM // P
    cw = const_width()
    cbuf = np.zeros((P, cw), np.float32)
    off = 0

    def put(block, cols):
        nonlocal off
        cbuf[: block.shape[0], off : off + cols] = block
        off += cols

    put(xT.reshape(nck, P, N).transpose(1, 0, 2).reshape(P, nck * N), nck * N)
    wqk = np.concatenate([wq2, wk2], axis=1)  # [dim, 128]
    put(wqk.reshape(nck, P, P).transpose(1, 0, 2).reshape(P, nck * P), nck * P)
    for w in (wg2, wv2):
        put(w.reshape(nck, P, 2 * DH).transpose(1, 0, 2).reshape(P, nck * 2 * DH),
            nck * 2 * DH)
    wout_cols = np.zeros((64 + DH, HPC * DIM), np.float32)
    for h in range(HPC):
        wout_cols[0:DH, h * DIM : (h + 1) * DIM] = wout2[h * DH : (h + 1) * DH, :]
        wout_cols[64 : 64 + DH, h * DIM : (h + 1) * DIM] = \
            wout2[h * DH : (h + 1) * DH, :]
    put(wout_cols, HPC * DIM)
    bg_cols = np.zeros((DH, HPC), np.float32)
    for h in range(HPC):
        bg_cols[:, h] = bg2[h * DH : (h + 1) * DH]
    put(bg_cols, HPC)
    put(np.eye(16, dtype=np.float32), 16)
    assert off == cw
    return cbuf


def shard_inputs(x, attn_bias, Wq, Wkv, Wg, bg, Wout):
    import ml_dtypes

    scale = DH ** -0.5
    in_maps = []
    for c in range(NCORES):
        b = c // 4
        hp = c % 4
        hs = slice(2 * hp * DH, (2 * hp + 2) * DH)
        cbuf = pack_consts(
            np.ascontiguousarray(x[b].T),
            Wq[:, hs] * np.float32(scale),
            Wkv[:, :DIM][:, hs],
            Wkv[:, DIM:][:, hs],
            Wg[:, hs],
            bg[hs].astype(np.float32),
            Wout[hs, :],
        )
        in_maps.append(
            {
                "cb": cbuf.astype(ml_dtypes.bfloat16),
                "biasT": np.exp(
                    attn_bias[b, 2 * hp : 2 * hp + 2].transpose(0, 2, 1)
                ).astype(ml_dtypes.bfloat16),
            }
        )
    return in_maps


def gather_outputs(outs, bout):
    parts = [np.asarray(o, np.float32).sum(axis=0) for o in outs]
    out0 = parts[0] + parts[1] + parts[2] + parts[3]
    out1 = parts[4] + parts[5] + parts[6] + parts[7]
    return (np.stack([out0, out1]) + bout).astype(np.float32)


def _numpy_fallback(x, mask, attn_bias, Wq, Wkv, Wg, bg, Wout, bout):
    b, n, dim = x.shape
    h, dh = HEADS, DH
    scale = dh ** -0.5
    q = (x @ Wq).reshape(b, n, h, dh).transpose(0, 2, 1, 3)
    kv = x @ Wkv
    k = kv[..., : h * dh].reshape(b, n, h, dh).transpose(0, 2, 1, 3)
    v = kv[..., h * dh :].reshape(b, n, h, dh).transpose(0, 2, 1, 3)
    dots = np.einsum("bhid,bhjd->bhij", q * scale, k) + attn_bias
    pair = mask[:, None, :, None] & mask[:, None, None, :]
    dots = np.where(pair, dots, -np.finfo(dots.dtype).max)
    dots -= dots.max(axis=-1, keepdims=True)
    attn = np.exp(dots)
    attn /= attn.sum(axis=-1, keepdims=True)
    out = np.einsum("bhij,bhjd->bhid", attn, v)
    out = out.transpose(0, 2, 1, 3).reshape(b, n, h * dh)
    gates = 1.0 / (1.0 + np.exp(-(x @ Wg + bg)))
    return ((out * gates) @ Wout + bout).astype(np.float32)


_NC_CACHE = {}


def _get_nc():
    if "nc" not in _NC_CACHE:
        _NC_CACHE["nc"] = build_nc()
    return _NC_CACHE["nc"]


def run_on_device(in_maps, **kwargs):
    from concourse.bass_utils import run_bass_kernel_spmd

    nc = _get_nc()
    return run_bass_kernel_spmd(nc, in_maps, core_ids=list(range(NCORES)), **kwargs)


def kernel(x, mask, attn_bias, Wq, Wkv, Wg, bg, Wout, bout):
    x = np.asarray(x, np.float32)
    mask = np.asarray(mask)
    attn_bias = np.asarray(attn_bias, np.float32)
    Wq = np.asarray(Wq, np.float32)
    Wkv = np.asarray(Wkv, np.float32)
    Wg = np.asarray(Wg, np.float32)
    bg = np.asarray(bg, np.float32)
    Wout = np.asarray(Wout, np.float32)
    bout = np.asarray(bout, np.float32)

    if not mask.all():
        return _numpy_fallback(x, mask, attn_bias, Wq, Wkv, Wg, bg, Wout, bout)

    in_maps = shard_inputs(x, attn_bias, Wq, Wkv, Wg, bg, Wout)
    res = run_on_device(in_maps)
    outs = [res.results[i]["out"] for i in range(NCORES)]
    return gather_outputs(outs, bout)


if __name__ == "__main__":
    nc = build_nc()
    bad = check_mm_waits(nc)
    print("multi-wait engine instructions:", len(bad))
    for b_ in bad[:30]:
        print("  ", b_)



# revision 27
# speedup vs baseline: 1.1675x; 1.1675x over previous
"""Trainium2 Bass kernel v2 for gated multi-head attention with additive bias.

Reference (b=2, n=2048, dim=256, h=8, dh=32):
    q = x @ Wq;  k,v = split(x @ Wkv);  dots = q k^T / sqrt(dh) + attn_bias
    attn = softmax(dots);  out = attn @ v
    out = out * sigmoid(x @ Wg + bg);  return out @ Wout + bout

Sharding: 16 (batch, head) pairs -> 8 cores, 2 heads each (core c handles
batch c//4, heads 2*(c%4), 2*(c%4)+1).  Host ships exp(bias)^T in bf16 and
sums the per-core partial outputs.

v2 design (vs the v1 baseline):
  * S^T chunks land in a manual 3-slot PSUM ring (each slot [128,1024] = 2
    banks).  exp() reads PAIRS of adjacent slots as one wide [128,2048] ACT
    instruction when the ring doesn't wrap (2 of 3 pairs) -- ACT is the
    bottleneck engine and its (N+352)-cycle cost rewards wide reads.
  * attn@v accumulates into ONE bank per i-half with M=33 blocks at psum
    base partitions 0 and 64 (tile_position col packing) -- 2 banks total,
    leaving 6 banks for the S ring.  The 33rd v-column of ones produces
    softmax row sums in psum rows 32/96.
  * q/k for both heads are computed with a single fused M=128 weight
    (columns = [q_h0|q_h1|k_h0|k_h1]), gates in the attn@v banks, so the PE
    stream is dense from t=0 (HAM stays un-throttled at 2.4 GHz).
  * Normalization deferred past Wout: row sums bounce through DRAM to
    become per-partition scalars [128, nt]; reciprocal multiplies the
    projection output.  Head 0's projection is interleaved into head 1's
    main loop using just-freed ring slots.

Toolchain: walrus accepts at most ONE semaphore wait per compute-engine
instruction; _split_multi_waits moves extras onto same-engine NOPs.
"""

import os
import sys

import numpy as np

for _p in ("/opt/trn_rl_repo", "/root/.axon_site/_ro/trn_rl_repo"):
    if os.path.isdir(_p) and _p not in sys.path:
        sys.path.insert(0, _p)

B = 2
N = 2048
DIM = 256
HEADS = 8
DH = 32
HPC = 2
NCORES = 8
P = 128
NT = N // P          # 16 j-tiles
NQ = N // 512        # 4 query chunks of 512


def build_nc(split_waits=True):
    import concourse.bass as bass
    import concourse.mybir as mybir
    from concourse.bass import ts
    from concourse.tile import TileContext

    f32 = mybir.dt.float32
    bf16 = mybir.dt.bfloat16
    Act = mybir.ActivationFunctionType

    n, dim, nt = N, DIM, NT
    nck = dim // P               # 2 contraction chunks over model dim
    cw = const_width()

    from concourse import tile_sem_assignment as _tsa

    _swdge_prev = _tsa.NUM_SWDGE_GLOBAL_SEMS

    nc = bass.Bass()

    cb = nc.declare_dram_parameter("cb", [P, cw], bf16, isOutput=False)
    biasT = nc.declare_dram_parameter("biasT", [HPC, n, n], bf16, isOutput=False)
    out_ext = nc.declare_dram_parameter("out", [HPC, n, dim], bf16, isOutput=True)

    _tsa.NUM_SWDGE_GLOBAL_SEMS = 1
    with TileContext(nc) as tc:
        with (
            tc.tile_pool(name="consts", bufs=1) as consts,
            tc.tile_pool(name="dram", bufs=2, space="DRAM") as dpool,
            tc.tile_pool(name="s_ps", bufs=3, space="PSUM") as spool,
            tc.tile_pool(name="av_ps", bufs=1, space="PSUM") as avpool,
            tc.tile_pool(name="bias", bufs=4) as bpool,
            tc.tile_pool(name="attn", bufs=7) as apool,
            tc.tile_pool(name="et", bufs=3) as etpool,
        ):
            # ---- single packed constant DMA ----
            cb_sb = consts.tile([P, cw], bf16, tag="cb", name="cb_sb")
            nc.sync.dma_start(out=cb_sb, in_=cb[:, :])
            off = 0

            def take(cols):
                nonlocal off
                ap = cb_sb[:, off : off + cols]
                off += cols
                return ap

            xT_sb = take(nck * n).rearrange("p (c n) -> p c n", c=nck)
            # per chunk c, cols = [q_h0|q_h1|k_h0|k_h1] x 32
            wqk = take(nck * P).rearrange("p (c m) -> p c m", c=nck)
            # gate weights: per chunk c, cols = [g_h0|g_h1] x 32
            wgg = take(nck * 2 * DH).rearrange("p (c m) -> p c m", c=nck)
            # v weights: per chunk c, cols = [v_h0|v_h1] x 32
            wvv = take(nck * 2 * DH).rearrange("p (c m) -> p c m", c=nck)
            # wout replicated on partition rows 0-31 and 64-95
            wout_cols = take(HPC * dim)
            wout_h = [wout_cols[0:DH, h * dim : (h + 1) * dim] for h in range(HPC)]
            wout_h_odd = [wout_cols[64 : 64 + DH, h * dim : (h + 1) * dim]
                          for h in range(HPC)]
            bg_cols = take(HPC)
            bg_h = [bg_cols[0:DH, h : h + 1] for h in range(HPC)]
            eye_cols = take(16)
            eye16 = eye_cols[0:16, :]
            assert off == cw

            # ---- persistent SBUF ----
            qT_h = [consts.tile([P, n], bf16, tag=f"qT{h}", name=f"qT{h}")
                    for h in range(HPC)]
            kT_h = [consts.tile([P, n], bf16, tag=f"kT{h}", name=f"kT{h}")
                    for h in range(HPC)]
            gT_h = [consts.tile([DH + 1, n], bf16, tag=f"gT{h}", name=f"gT{h}")
                    for h in range(HPC)]
            gT_odd = [consts.tile([97, n], bf16, tag=f"gTo{h}", name=f"gTo{h}")
                      for h in range(HPC)]
            vtmp = consts.tile([P, nt, 2 * DH], bf16, tag="vtmp", name="vtmp")
            # per head: [v_h | ones] (33 cols) per j-tile
            vaug = consts.tile([P, nt, HPC, DH + 1], bf16, tag="vaug", name="vaug")
            gatedT_h = [consts.tile([DH + 1, n], bf16, tag=f"gatedT{h}",
                                    name=f"gatedT{h}") for h in range(HPC)]
            godT_h = [consts.tile([97, n], bf16, tag=f"godT{h}", name=f"godT{h}")
                      for h in range(HPC)]
            recip_h = [consts.tile([P, nt], f32, tag=f"recip{h}",
                                   name=f"recip{h}") for h in range(HPC)]
            o_all = [consts.tile([P, nt, dim], bf16, tag=f"o{h}", name=f"o{h}")
                     for h in range(HPC)]

            zrow = consts.tile([1, 512], bf16, tag="zrow", name="zrow")
            nc.vector.memset(zrow, 0.0)

            # ---- PSUM: 3-slot S pool (6 banks) + attn@v banks (2) ----
            SPAD = [P, 1024]
            av = avpool.tile([P, 1024], f32, tag="av", name="av")



            # =========== prologue ===========
            # q/k: col-tiled pair per strip (q_h0 -> rows 0-31, q_h1 ->
            # rows 32-63 concurrently), evacuated as one aligned [64, .] copy
            def emit_qk(i, half):
                # i: 0 = q (heads 0/1), 1 = k (heads 0/1).  The two heads go
                # to different pool slots (distinct banks) at col groups 0/1,
                # so the matmul pairs run concurrently on the PE.
                qkA = spool.tile([DH, 1024], f32, tag="s", name="qkA",
                                 padded_shape=SPAD)
                qkB = spool.tile([2 * DH, 1024], f32, tag="s", name="qkB",
                                 padded_shape=SPAD)
                dsts = (qT_h, kT_h)[i]
                for c in range(nck):
                    for s in range(2):
                        for e in range(2):
                            out = (qkA[0:DH, ts(s, 512)] if e == 0
                                   else qkB[DH : 2 * DH, ts(s, 512)])
                            nc.tensor.matmul(
                                out,
                                wqk[:, c, (2 * i + e) * DH :
                                    (2 * i + e + 1) * DH],
                                xT_sb[:, c, half * 1024 + s * 512 :
                                      half * 1024 + (s + 1) * 512],
                                start=(c == 0),
                                stop=(c == nck - 1),
                            )
                cols = slice(half * 1024, (half + 1) * 1024)
                nc.vector.tensor_copy(dsts[0][0:DH, cols], qkA[0:DH, :])
                nc.vector.tensor_copy(dsts[1][DH : 2 * DH, cols],
                                      qkB[DH : 2 * DH, :])

            def replicate_qk(h):
                # fill the remaining 3 row groups of each [128, n] tile
                nc.sync.dma_start(out=qT_h[h][DH : 2 * DH, :] if h == 0
                                  else qT_h[h][0:DH, :],
                                  in_=qT_h[h][0:DH, :] if h == 0
                                  else qT_h[h][DH : 2 * DH, :])
                nc.sync.dma_start(out=kT_h[h][DH : 2 * DH, :] if h == 0
                                  else kT_h[h][0:DH, :],
                                  in_=kT_h[h][0:DH, :] if h == 0
                                  else kT_h[h][DH : 2 * DH, :])
                for tile in (qT_h[h], kT_h[h]):
                    nc.sync.dma_start(out=tile[2 * DH : 4 * DH, :],
                                      in_=tile[0 : 2 * DH, :])

            def emit_v(t):
                vps = spool.tile([P, 64], f32, tag="s", name="vps",
                                 padded_shape=SPAD)
                for c in range(nck):
                    nc.tensor.matmul(
                        vps,
                        xT_sb[:, c, ts(t, P)],
                        wvv[:, c, :],
                        start=(c == 0),
                        stop=(c == nck - 1),
                    )
                nc.vector.tensor_copy(vtmp[:, t, :], vps)

            # q/k first: the main loop's S matmuls gate on these
            for i in (0, 1):
                for half in range(2):
                    emit_qk(i, half)
            replicate_qk(0)
            replicate_qk(1)
            # gates: [32, 512] psums; sigmoids run on ACT before the first exp
            for h in range(HPC):
                for s in range(4):
                    gps = spool.tile([DH, 512], f32, tag="s", name="gps",
                                     padded_shape=SPAD)
                    for c in range(nck):
                        nc.tensor.matmul(
                            gps,
                            wgg[:, c, h * DH : (h + 1) * DH],
                            xT_sb[:, c, ts(s, 512)],
                            start=(c == 0),
                            stop=(c == nck - 1),
                        )
                    nc.scalar.activation(
                        out=gT_h[h][0:DH, ts(s, 512)],
                        in_=gps,
                        func=Act.Sigmoid,
                        scale=1.0,
                        bias=bg_h[h],
                    )
            for h in range(HPC):
                nc.vector.memset(gT_h[h][DH : DH + 1, :], 1.0)
                nc.sync.dma_start(out=gT_odd[h][64:97, :], in_=gT_h[h][0:33, :])

            # =========== main loops ===========
            def emit_proj(h, t):
                """Projection for (head h, i-tile t) through a transient psum
                strip; normalized output lands in o_all[h]."""
                pp = spool.tile([P, dim], f32, tag="s", name="pp",
                                padded_shape=SPAD)
                if (t // 4) % 2 == 0:
                    lhsT = gatedT_h[h][0:DH, ts(t, P)]
                    rhs = wout_h[h]
                else:
                    lhsT = godT_h[h][64 : 64 + DH, ts(t, P)]
                    rhs = wout_h_odd[h]
                nc.tensor.matmul(pp, lhsT, rhs, start=True, stop=True)
                nc.vector.tensor_scalar_mul(
                    o_all[h][:, t, :], pp, recip_h[h][:, t : t + 1]
                )

            PROJ_ORDER = [0, 4, 1, 5, 2, 6, 3, 7, 8, 12, 9, 13, 10, 14,
                          11, 15]
            cc = 0
            for h in range(HPC):
                # open the two attn@v banks lazily at the first AV flush:
                # K=1 zero outer-products write all 128 partitions with
                # start=True.  Deferred evacuation reads of the PREVIOUS
                # head's results must precede these writes in emission order.
                opened = [False]

                def open_banks():
                    for bank in range(2):
                        nc.tensor.matmul(
                            av[:, 512 * bank : 512 * bank + 512],
                            zrow[0:1, 0:P],
                            zrow[0:1, 0:512],
                            start=True,
                            stop=False,
                            skip_group_check=True,
                        )
                    opened[0] = True
                def emit_av(jc_, m_, attn_):
                    # attn@v: M=33 blocks at psum partitions 0/64 of bank m_
                    for e in range(2):
                        nc.tensor.matmul(
                            av[64 * e : 64 * e + DH + 1,
                               512 * m_ : 512 * m_ + 512],
                            vaug[:, jc_, h, :],
                            attn_[:, ts(e, 512)],
                            start=False,
                            stop=(jc_ == nt - 1 and e == 1),
                            skip_group_check=True,
                        )

                pend = []
                for jc in range(nt):
                    bt = bpool.tile([P, n], bf16, tag="bias", name="bt")
                    nc.sync.dma_start(out=bt, in_=biasT[h, ts(jc, P), :])
                    # two 1024-wide chunks per j-tile; AV emission lags two
                    # chunks so the in-order PE queue never waits on exp/mul
                    for m in range(2):
                        sps = spool.tile([P, 1024], f32, tag="s", name="sps")
                        for s in range(2):
                            g = (2 * cc + s) % 4
                            nc.tensor.matmul(
                                sps[:, ts(s, 512)],
                                kT_h[h][32 * g : 32 * (g + 1), ts(jc, P)],
                                qT_h[h][32 * g : 32 * (g + 1),
                                        m * 1024 + s * 512 :
                                        m * 1024 + (s + 1) * 512],
                                start=True,
                                stop=True,
                                tile_position=(32 * g, 0),
                            )
                        cc += 1
                        et = etpool.tile([P, 1024], bf16, tag="et", name="et")
                        nc.scalar.activation(out=et, in_=sps, func=Act.Exp)
                        attn = apool.tile([P, 1024], bf16, tag="attn",
                                          name="attn")
                        nc.vector.tensor_mul(
                            attn, et, bt[:, m * 1024 : (m + 1) * 1024]
                        )
                        if h == 0 and jc == 0 and m == 0:
                            # v + vaug assembly rides the pipe-fill bubble:
                            # emitted before AV(0) enters the in-order PE queue
                            nc.vector.memset(vaug[:, :, :, DH : DH + 1], 1.0)
                            for t in range(nt):
                                emit_v(t)
                            nc.vector.tensor_copy(
                                vaug[:, :, :, 0:DH],
                                vtmp.rearrange("p t (h d) -> p t h d", h=HPC),
                            )
                        pend.append((jc, m, attn))
                    # flush attn@v two chunks at a time: consecutive same-type
                    # matmul runs keep LDWEIGHTS hidden under the streams
                    while len(pend) > 4:
                        if not opened[0]:
                            open_banks()
                        emit_av(*pend.pop(0))
                        emit_av(*pend.pop(0))
                    if h == 1 and jc <= 1:
                        deferred_gated(2 * jc)
                        deferred_gated(2 * jc + 1)
                    if h == 1 and jc in (2, 3):
                        deferred_strip(2 * (jc - 2))
                        deferred_strip(2 * (jc - 2) + 1)
                    if h == 1 and jc == 4:
                        deferred_recip()
                    # interleave head-0 projection into head-1's loop
                    # (order alternates row groups 0/2 so proj pairs overlap)
                    if h == 1 and 5 <= jc:
                        t2 = PROJ_ORDER[jc - 5]
                        emit_proj(0, t2)
                        nc.sync.dma_start(
                            out=out_ext[0, t2 * P : (t2 + 1) * P, :],
                            in_=o_all[0][:, t2, :],
                        )
                for a in pend:
                    emit_av(*a)

                # ---- evacuate attn@v psum for this head ----
                def emit_gated(q, h_=h):
                    qcol = 512 * (q // 2)
                    if q % 2 == 0:
                        nc.vector.tensor_mul(
                            gatedT_h[h_][:, ts(q, 512)],
                            av[0 : DH + 1, qcol : qcol + 512],
                            gT_h[h_][:, ts(q, 512)],
                        )
                    else:
                        nc.vector.tensor_mul(
                            godT_h[h_][64:97, ts(q, 512)],
                            av[64 : 64 + DH + 1, qcol : qcol + 512],
                            gT_odd[h_][64:97, ts(q, 512)],
                        )

                if h == 0:
                    deferred_gated = emit_gated
                else:
                    for q in range(NQ):
                        emit_gated(q)
                # ones-row of gates turned the psum sums rows into row DH of
                # the gated tiles; bounce through DRAM to [128, nt]
                scr = dpool.tile([n], bf16, tag="scr", name="scr")

                def emit_strip(q, h_=h, scr_=scr):
                    if q % 2 == 0:
                        strip = gatedT_h[h_][DH : DH + 1, ts(q, 512)]
                    else:
                        strip = godT_h[h_][96:97, ts(q, 512)]
                    nc.sync.dma_start(out=scr_[ts(q, 512)], in_=strip)

                if h == 0:
                    deferred_strip = emit_strip
                else:
                    for q in range(NQ):
                        emit_strip(q)
                def emit_recip(h_=h, scr_=scr):
                    sums16 = consts.tile([nt, P], bf16, tag=f"sums16{h_}",
                                         name=f"sums16{h_}")
                    nc.sync.dma_start(
                        out=sums16, in_=scr_[:].rearrange("(t p) -> t p", p=P)
                    )
                    spt = spool.tile([P, nt], f32, tag="s", name="spt",
                                     padded_shape=SPAD)
                    nc.tensor.matmul(spt, sums16, eye16, start=True, stop=True)
                    nc.vector.reciprocal(recip_h[h_], spt)

                if h == 0:
                    # defer: keeps the transpose matmul out of the in-order PE
                    # queue ahead of head 1's S chunks
                    deferred_recip = emit_recip
                else:
                    emit_recip()

            # remaining projections + head-1 epilogue
            for k in range(nt - 5, nt):
                t2 = PROJ_ORDER[k]
                emit_proj(0, t2)
                nc.sync.dma_start(
                    out=out_ext[0, t2 * P : (t2 + 1) * P, :],
                    in_=o_all[0][:, t2, :],
                )
            for t in range(nt):
                t2 = PROJ_ORDER[t]
                emit_proj(1, t2)
                nc.sync.dma_start(
                    out=out_ext[1, t2 * P : (t2 + 1) * P, :],
                    in_=o_all[1][:, t2, :],
                )

    _tsa.NUM_SWDGE_GLOBAL_SEMS = _swdge_prev
    if split_waits:
        _split_multi_waits(nc)
    return nc


def _split_multi_waits(nc):
    """walrus accepts at most ONE semaphore wait per engine instruction;
    extra waits ride same-engine NOPs inserted just before (queues execute
    in order)."""
    import concourse.mybir as mybir

    n = 0
    for f in nc.m.functions:
        for blk in f.blocks:
            out = []
            changed = False
            for inst in blk.instructions:
                si = getattr(inst, "sync_info", None)
                ws = list(si.on_wait) if si and si.on_wait else []
                if len(ws) > 1:
                    for w in ws[:-1]:
                        nop = mybir.InstNoOp(
                            name=f"I-waitsplit-{n}",
                            engine=inst.engine,
                            sync_info=mybir.SyncInfo(on_wait=[w], on_update=[]),
                        )
                        out.append(nop)
                        n += 1
                    si.on_wait = [ws[-1]]
                    inst.sync_info = si
                    changed = True
                out.append(inst)
            if changed:
                blk.instructions = out


def check_mm_waits(nc):
    bad = []
    for f in nc.m.functions:
        for blk in f.blocks:
            for inst in blk.instructions:
                if type(inst).__name__ in ("InstDMACopy", "InstDrain"):
                    continue
                si = getattr(inst, "sync_info", None)
                ws = list(si.on_wait) if si and si.on_wait else []
                if len(ws) > 1:
                    bad.append(
                        (inst.name, type(inst).__name__,
                         [(w.ant_name, w.wait_value) for w in ws])
                    )
    return bad


def const_width():
    nck = DIM // P
    return (nck * N + nck * P + nck * 2 * DH + nck * 2 * DH + HPC * DIM
            + HPC + 16)


def pack_consts(xT, wq2, wk2, wv2, wg2, bg2, wout2):
    """xT [dim, n]; wq2/wk2/wv2/wg2 [dim, 2*DH] (head-major cols);
    bg2 [2*DH]; wout2 [2*DH, dim]."""
    nck = DIM // P
    cw = const_width()
    cbuf = np.zeros((P, cw), np.float32)
    off = 0

    def put(block, cols):
        nonlocal off
        cbuf[: block.shape[0], off : off + cols] = block
        off += cols

    put(xT.reshape(nck, P, N).transpose(1, 0, 2).reshape(P, nck * N), nck * N)
    wqk = np.concatenate([wq2, wk2], axis=1)  # [dim, 128]
    put(wqk.reshape(nck, P, P).transpose(1, 0, 2).reshape(P, nck * P), nck * P)
    for w in (wg2, wv2):
        put(w.reshape(nck, P, 2 * DH).transpose(1, 0, 2).reshape(P, nck * 2 * DH),
            nck * 2 * DH)
    wout_cols = np.zeros((64 + DH, HPC * DIM), np.float32)
    for h in range(HPC):
        wout_cols[0:DH, h * DIM : (h + 1) * DIM] = wout2[h * DH : (h + 1) * DH, :]
        wout_cols[64 : 64 + DH, h * DIM : (h + 1) * DIM] = \
            wout2[h * DH : (h + 1) * DH, :]
    put(wout_cols, HPC * DIM)
    bg_cols = np.zeros((DH, HPC), np.float32)
    for h in range(HPC):
        bg_cols[:, h] = bg2[h * DH : (h + 1) * DH]
    put(bg_cols, HPC)
    put(np.eye(16, dtype=np.float32), 16)
    assert off == cw
    return cbuf


def shard_inputs(x, attn_bias, Wq, Wkv, Wg, bg, Wout):
    import ml_dtypes

    scale = DH ** -0.5
    in_maps = []
    for c in range(NCORES):
        b = c // 4
        hp = c % 4
        hs = slice(2 * hp * DH, (2 * hp + 2) * DH)
        cbuf = pack_consts(
            np.ascontiguousarray(x[b].T),
            Wq[:, hs] * np.float32(scale),
            Wkv[:, :DIM][:, hs],
            Wkv[:, DIM:][:, hs],
            Wg[:, hs],
            bg[hs].astype(np.float32),
            Wout[hs, :],
        )
        in_maps.append(
            {
                "cb": cbuf.astype(ml_dtypes.bfloat16),
                "biasT": np.exp(
                    attn_bias[b, 2 * hp : 2 * hp + 2].transpose(0, 2, 1)
                ).astype(ml_dtypes.bfloat16),
            }
        )
    return in_maps


def gather_outputs(outs, bout):
    parts = [np.asarray(o, np.float32).sum(axis=0) for o in outs]
    out0 = parts[0] + parts[1] + parts[2] + parts[3]
    out1 = parts[4] + parts[5] + parts[6] + parts[7]
    return (np.stack([out0, out1]) + bout).astype(np.float32)


def _numpy_fallback(x, mask, attn_bias, Wq, Wkv, Wg, bg, Wout, bout):
    b, n, dim = x.shape
    h, dh = HEADS, DH
    scale = dh ** -0.5
    q = (x @ Wq).reshape(b, n, h, dh).transpose(0, 2, 1, 3)
    kv = x @ Wkv
    k = kv[..., : h * dh].reshape(b, n, h, dh).transpose(0, 2, 1, 3)
    v = kv[..., h * dh :].reshape(b, n, h, dh).transpose(0, 2, 1, 3)
    dots = np.einsum("bhid,bhjd->bhij", q * scale, k) + attn_bias
    pair = mask[:, None, :, None] & mask[:, None, None, :]
    dots = np.where(pair, dots, -np.finfo(dots.dtype).max)
    dots -= dots.max(axis=-1, keepdims=True)
    attn = np.exp(dots)
    attn /= attn.sum(axis=-1, keepdims=True)
    out = np.einsum("bhij,bhjd->bhid", attn, v)
    out = out.transpose(0, 2, 1, 3).reshape(b, n, h * dh)
    gates = 1.0 / (1.0 + np.exp(-(x @ Wg + bg)))
    return ((out * gates) @ Wout + bout).astype(np.float32)


_NC_CACHE = {}


def _get_nc():
    if "nc" not in _NC_CACHE:
        _NC_CACHE["nc"] = build_nc()
    return _NC_CACHE["nc"]


def run_on_device(in_maps, **kwargs):
    from concourse.bass_utils import run_bass_kernel_spmd

    nc = _get_nc()
    return run_bass_kernel_spmd(nc, in_maps, core_ids=list(range(NCORES)), **kwargs)


def kernel(x, mask, attn_bias, Wq, Wkv, Wg, bg, Wout, bout):
    x = np.asarray(x, np.float32)
    mask = np.asarray(mask)
    attn_bias = np.asarray(attn_bias, np.float32)
    Wq = np.asarray(Wq, np.float32)
    Wkv = np.asarray(Wkv, np.float32)
    Wg = np.asarray(Wg, np.float32)
    bg = np.asarray(bg, np.float32)
    Wout = np.asarray(Wout, np.float32)
    bout = np.asarray(bout, np.float32)

    if not mask.all():
        return _numpy_fallback(x, mask, attn_bias, Wq, Wkv, Wg, bg, Wout, bout)

    in_maps = shard_inputs(x, attn_bias, Wq, Wkv, Wg, bg, Wout)
    res = run_on_device(in_maps)
    outs = [res.results[i]["out"] for i in range(NCORES)]
    return gather_outputs(outs, bout)


if __name__ == "__main__":
    nc = build_nc()
    bad = check_mm_waits(nc)
    print("multi-wait engine instructions:", len(bad))
    for b_ in bad[:30]:
        print("  ", b_)

